# revision 19
# baseline (speedup 1.0000x reference)
"""Trainium2 Bass kernel for nn_BOW (EmbeddingBag + MLP + BatchNorm + sigmoid).

reference:
    gathered = emb[tokens]                               # [T, H]
    pooled   = segment_mean(gathered, segment_ids, B)    # [B, H]
    x = pooled @ W1.T + b1                               # [B, H]
    x = batchnorm_train(x, gamma, beta)                  # batch stats
    x = relu(x)
    out = sigmoid(x @ W2.T + b2)                         # [B, 1]

Sharding: data-parallel over 8 cores; core c owns segments
[c*B/8, (c+1)*B/8) (segments are contiguous in the sorted segment_ids).
Weights are replicated. BatchNorm batch statistics are combined with a
4 KB AllGather.

v2 design (per core):
  - The embedding table is converted to bf16 on the host and shipped as
    an f32-typed [V, 256] table (bf16 pairs packed per f32 word), so each
    dma_gather row moves 1 KB instead of 2 KB.  Gathered tiles are
    bitcast back to bf16 in SBUF.
  - Selection matrices S'[p, s*gt + k] = (segsc[p, tile k] == s) are
    built on DVE in bf16 with both operands packed along the innermost
    (tile) axis, which qualifies for the DVE 2x perf mode.  The matmul
    reads S' tiles through a strided AP (stride gt over segments).
  - psum[seg, feat] accumulated per 128-segment block via bf16 matmuls
    (S'_k.T @ G_k), scaled by 1/count on the Activation engine
    (per-partition scale), transposed by PE, and stashed as bf16.
  - fc1 runs per (block, jc): 128-col matmuls; BN batch stats (sum and
    sum-of-squares) accumulate per block from PSUM (DVE reduce + ACT
    Square-accumulate), so only the last block's stats sit on the
    critical tail.
  - b1 is algebraically dropped (BatchNorm cancels the fc1 bias).
  - Batch stats combined with one 4 KB AllGather; BN scale/shift are
    computed per feature and applied fused with ReLU on ACT; fc2 +
    sigmoid finish on PE/ACT.
"""
import os
import sys

sys.path.insert(0, "/opt/trn_rl_repo")

import numpy as np
import ml_dtypes

import concourse.bass as bass
import concourse.mybir as mybir
import concourse.tile as tile
from concourse import bacc, bass_utils

F32 = mybir.dt.float32
F32R = mybir.dt.float32r
BF16 = mybir.dt.bfloat16
I16 = mybir.dt.int16
I32 = mybir.dt.int32

NCORES = 8
V = 100000
H = 512
HPW = H // 2                 # packed f32 words per emb row (bf16 pairs)
B = 4096
BN_EPS = 1e-5
NCHUNK = 4                   # vocab chunks (int16 gather index range)
CHUNK = V // NCHUNK          # 25000 rows per chunk
SEGS_PER_CORE = B // NCORES  # 512
NSB = SEGS_PER_CORE // 128   # 4 seg-blocks of 128 segments
GRAN = int(os.environ.get("K_GRAN", "2048"))  # max tokens per granule
GTM = GRAN // 128            # max tiles per granule
JC = H // 128                # 4 feature chunks


def _plan(tokens, segment_ids):
    """Host integer preprocessing: shard + reorder + pad token indices.

    Token order per core: blocks of 128 segments (sb), within a block 4
    vocab chunks, within a (sb, chunk) run sorted by token id for HBM
    locality.  Runs are padded to multiples of 128 with token 0 /
    segment -1 and padded to the max length across cores so all cores
    compile to the same program.
    """
    tokens = np.asarray(tokens).astype(np.int64)
    segment_ids = np.asarray(segment_ids).astype(np.int64)

    seg_start = np.searchsorted(segment_ids, np.arange(B + 1))
    chunk_of = np.minimum(tokens // CHUNK, NCHUNK - 1).astype(np.int64)

    runs = [[[None] * NCHUNK for _ in range(NSB)] for _ in range(NCORES)]
    for c in range(NCORES):
        for sb in range(NSB):
            lo = seg_start[c * SEGS_PER_CORE + sb * 128]
            hi = seg_start[c * SEGS_PER_CORE + (sb + 1) * 128]
            tk = tokens[lo:hi]
            sg = segment_ids[lo:hi] - (c * SEGS_PER_CORE + sb * 128)
            ck = chunk_of[lo:hi]
            for ch in range(NCHUNK):
                m = ck == ch
                tkm, sgm = tk[m] - ch * CHUNK, sg[m]
                order = np.argsort(tkm, kind="stable")
                runs[c][sb][ch] = (tkm[order], sgm[order])

    L = np.zeros((NSB, NCHUNK), np.int64)
    for sb in range(NSB):
        for ch in range(NCHUNK):
            mx = max(len(runs[c][sb][ch][0]) for c in range(NCORES))
            L[sb, ch] = ((mx + 127) // 128) * 128 if mx > 0 else 0

    gsizes = {}
    for sb in range(NSB):
        for ch in range(NCHUNK):
            n, out = L[sb, ch], []
            while n > 0:
                g = min(n, GRAN)
                out.append(int(g))
                n -= g
            gsizes[(sb, ch)] = out

    ntiles_total = int(L.sum()) // 128
    idx_cols = int(L.sum()) // 16

    per_core = []
    for c in range(NCORES):
        idx16 = np.zeros((16, idx_cols), np.int16)
        segsc = np.full((128, ntiles_total), -1.0, ml_dtypes.bfloat16)
        recip = np.zeros((128, NSB), np.float32)
        col = 0
        tcol = 0
        for sb in range(NSB):
            for ch in range(NCHUNK):
                idx, sga = runs[c][sb][ch]
                Lr = int(L[sb, ch])
                if Lr == 0:
                    continue
                pi = np.zeros(Lr, np.int16)
                pi[: len(idx)] = idx
                ps = np.full(Lr, -1.0, np.float32)
                ps[: len(sga)] = sga
                off = 0
                for g in gsizes[(sb, ch)]:
                    blk = pi[off:off + g]
                    idx16[:, col:col + g // 16] = blk.reshape(-1, 16).T
                    col += g // 16
                    off += g
                segsc[:, tcol:tcol + Lr // 128] = (
                    ps.reshape(-1, 128).T.astype(ml_dtypes.bfloat16))
                tcol += Lr // 128
            cnt = np.bincount(
                segment_ids[seg_start[c * SEGS_PER_CORE + sb * 128]:
                            seg_start[c * SEGS_PER_CORE + (sb + 1) * 128]]
                - (c * SEGS_PER_CORE + sb * 128),
                minlength=128,
            ).astype(np.float32)
            recip[:, sb] = 1.0 / np.maximum(cnt, 1.0)
        idx16 = np.tile(idx16, (8, 1))     # replicate for the 8 Q7 cores
        # recipT[p, sb*128 + s] = 1/count(segment s of block sb), all p
        recipT = np.tile(recip.T.reshape(1, -1), (128, 1)).astype(np.float32)
        per_core.append({"idx16": idx16, "segsc": segsc, "recipT": recipT})

    return L, gsizes, ntiles_total, idx_cols, per_core


def _build(L, gsizes, ntiles_total, idx_cols):
    nc = bacc.Bacc(None, num_devices=NCORES, num_swdge_queues=4)

    embp = nc.dram_tensor("embp", [V, HPW], F32, kind="ExternalInput")
    idx16_d = nc.dram_tensor("idx16", [128, idx_cols], I16, kind="ExternalInput")
    segsc_d = nc.dram_tensor("segsc", [128, ntiles_total], BF16,
                             kind="ExternalInput")
    recip_d = nc.dram_tensor("recipT", [128, SEGS_PER_CORE], F32,
                             kind="ExternalInput")
    w1t_d = nc.dram_tensor("w1t", [128, JC * H], BF16, kind="ExternalInput")
    w2t_d = nc.dram_tensor("w2t", [128, JC], BF16, kind="ExternalInput")
    gb_d = nc.dram_tensor("gb", [128, 2 * JC], F32, kind="ExternalInput")
    b2_d = nc.dram_tensor("b2", [1, 1], F32, kind="ExternalInput")
    out_d = nc.dram_tensor("out", [1, SEGS_PER_CORE], F32, kind="ExternalOutput")

    with tile.TileContext(nc) as tc:
        with (
            tc.tile_pool(name="const", bufs=1) as constp,
            tc.tile_pool(name="gpool", bufs=int(os.environ.get("K_GBUFS", "3"))) as gpool,
            tc.tile_pool(name="spool", bufs=3) as spool,
            tc.tile_pool(name="work", bufs=2) as work,
            tc.tile_pool(name="ppool", bufs=1, space="PSUM") as ppool,
            tc.tile_pool(name="pfc", bufs=2, space="PSUM") as pfc,
            tc.tile_pool(name="pout", bufs=1, space="PSUM") as pout,
            tc.tile_pool(name="dram", bufs=1, space="DRAM") as dram,
        ):
            # --- constants / small loads ---
            idx16_sb = constp.tile([128, idx_cols], I16)
            g0cols = min(GRAN // 16, idx_cols)
            nc.sync.dma_start(out=idx16_sb[:, :g0cols], in_=idx16_d[:, :g0cols])
            if g0cols < idx_cols:
                nc.sync.dma_start(out=idx16_sb[:, g0cols:],
                                  in_=idx16_d[:, g0cols:])
            segsc_sb = constp.tile([128, ntiles_total], BF16)
            nc.sync.dma_start(out=segsc_sb[:], in_=segsc_d[:, :])
            recipT_sb = constp.tile([128, SEGS_PER_CORE], F32)
            nc.sync.dma_start(out=recipT_sb[:], in_=recip_d[:, :])
            w1t_sb = constp.tile([128, JC * H], BF16)
            nc.sync.dma_start(out=w1t_sb[:], in_=w1t_d[:, :])
            w2t_sb = constp.tile([128, JC], BF16)
            nc.sync.dma_start(out=w2t_sb[:], in_=w2t_d[:, :])
            gb_sb = constp.tile([128, 2 * JC], F32)
            nc.sync.dma_start(out=gb_sb[:], in_=gb_d[:, :])
            b2_sb = constp.tile([1, 1], F32)
            nc.sync.dma_start(out=b2_sb[:], in_=b2_d[:, :])

            # iota_exp[p, s*GTM + k] = s (bf16), for the S' equality build
            iota_i = constp.tile([128, 128 * GTM], I32)
            nc.gpsimd.iota(iota_i[:].rearrange("p (s k) -> p s k", s=128),
                           pattern=[[1, 128], [0, GTM]], base=0,
                           channel_multiplier=0)
            iota_exp = constp.tile([128, 128 * GTM], BF16)
            nc.vector.tensor_copy(out=iota_exp[:], in_=iota_i[:])

            # persistent activations
            pooledT = constp.tile([128, JC * SEGS_PER_CORE], BF16)
            xT = constp.tile([128, JC * SEGS_PER_CORE], BF16)
            yT = constp.tile([128, JC * SEGS_PER_CORE], BF16)
            sxA = constp.tile([128, NSB * JC], F32)    # col = sb*JC + jc
            sxxA = constp.tile([128, NSB * JC], F32)
            t01 = constp.tile([128, 2 * JC], F32)      # sb0+sb1 partial
            t012 = constp.tile([128, 2 * JC], F32)     # sb0+sb1+sb2 partial
            stats8 = constp.tile([128, 2 * JC], F32)   # sx | sxx

            # --- main loop: gather + transposed segment-sum + fc1/stats ---
            # psum[:, hc*128 + s] accumulates pooledT chunk hc for the
            # block's 128 segments:  psum_hc = G_hc.T @ S'  ([feat, seg]).
            tcol = 0
            icol = 0
            gq = 0
            HC4 = HPW // JC                 # 64 packed f32 words per chunk
            for sb in range(NSB):
                # one PSUM bank per hc chunk: group hc lives at cols
                # [hc*512, hc*512+128) so accumulation groups don't share
                # a zero region.
                psum = ppool.tile([128, JC * 512], F32, tag="seg")
                sb_tiles = int(L[sb].sum()) // 128
                done = 0
                for ch in range(NCHUNK):
                    for g in gsizes[(sb, ch)]:
                        gt = g // 128
                        G = gpool.tile([128, GTM * HPW], F32, tag="G")
                        nc.gpsimd.dma_gather(
                            out_ap=G[:, : gt * HPW].rearrange(
                                "p (k h) -> p k h", k=gt),
                            in_ap=embp[ch * CHUNK:(ch + 1) * CHUNK, :],
                            idxs_ap=idx16_sb[:, icol:icol + g // 16],
                            num_idxs=g,
                            num_idxs_reg=g,
                            elem_size=HPW,
                            queue_num=gq % 4,
                            single_packet=False,
                        )
                        gq += 1
                        icol += g // 16
                        # S'[p, s*gt + k] = (segsc[p, tcol+k] == s), bf16.
                        # Both operands packed innermost -> DVE 2x mode.
                        S = spool.tile([128, 128 * GTM], BF16, tag="S")
                        nc.vector.tensor_tensor(
                            out=S[:, : 128 * gt].rearrange(
                                "p (s k) -> p s k", s=128),
                            in0=iota_exp[:].rearrange(
                                "p (s k) -> p s k", s=128)[:, :, :gt],
                            in1=segsc_sb[:, tcol:tcol + gt].unsqueeze(1)
                                .broadcast_to([128, 128, gt]),
                            op=mybir.AluOpType.is_equal,
                        )
                        Sv = S[:, : 128 * gt].rearrange("p (s k) -> p s k",
                                                        s=128)
                        for t in range(gt):
                            for hc in range(JC):
                                nc.tensor.matmul(
                                    out=psum[:, hc * 512:hc * 512 + 128],
                                    lhsT=G[:, t * HPW + hc * HC4:
                                           t * HPW + (hc + 1) * HC4
                                           ].bitcast(BF16),
                                    rhs=Sv[:, :, t],
                                    start=(done == 0),
                                    stop=(done == sb_tiles - 1),
                                )
                            done += 1
                        tcol += gt

                # pooledT block = psum * (1/count), one DVE op
                nc.vector.tensor_tensor(
                    out=pooledT[:].rearrange("p (h s) -> p h s", h=JC)
                        [:, :, sb * 128:(sb + 1) * 128],
                    in0=psum[:].rearrange("p (h q) -> p h q", h=JC)
                        [:, :, :128],
                    in1=recipT_sb[:, sb * 128:(sb + 1) * 128].unsqueeze(1)
                        .broadcast_to([128, JC, 128]),
                    op=mybir.AluOpType.mult,
                )

                # fc1 for this block + partial batch stats from PSUM
                for jc in range(JC):
                    px = pfc.tile([128, 128], F32, tag="px")
                    for hc in range(JC):
                        nc.tensor.matmul(
                            out=px[:],
                            lhsT=w1t_sb[:, hc * H + jc * 128:
                                        hc * H + (jc + 1) * 128],
                            rhs=pooledT[:, hc * SEGS_PER_CORE + sb * 128:
                                        hc * SEGS_PER_CORE + (sb + 1) * 128],
                            start=(hc == 0), stop=(hc == JC - 1),
                        )

                    nc.vector.reduce_sum(
                        out=sxA[:, sb * JC + jc:sb * JC + jc + 1],
                        in_=px[:], axis=mybir.AxisListType.X)
                    sq = work.tile([128, 128], F32, tag="sq")
                    nc.scalar.activation(
                        out=sq[:], in_=px[:],
                        func=mybir.ActivationFunctionType.Square,
                        accum_out=sxxA[:, sb * JC + jc:sb * JC + jc + 1],
                    )
                    nc.scalar.activation(
                        out=xT[:, jc * SEGS_PER_CORE + sb * 128:
                               jc * SEGS_PER_CORE + (sb + 1) * 128],
                        in_=px[:],
                        func=mybir.ActivationFunctionType.Copy,
                    )
                # incremental stats folds: keep only the final 2 adds on
                # the tail.  fold01 after sb=1, fold012 after sb=2.
                if sb == 1:
                    nc.vector.tensor_tensor(
                        out=t01[:, :JC], in0=sxA[:, 0:JC],
                        in1=sxA[:, JC:2 * JC], op=mybir.AluOpType.add)
                    nc.vector.tensor_tensor(
                        out=t01[:, JC:], in0=sxxA[:, 0:JC],
                        in1=sxxA[:, JC:2 * JC], op=mybir.AluOpType.add)
                elif sb == 2:
                    nc.vector.tensor_tensor(
                        out=t012[:, :JC], in0=t01[:, :JC],
                        in1=sxA[:, 2 * JC:3 * JC], op=mybir.AluOpType.add)
                    nc.vector.tensor_tensor(
                        out=t012[:, JC:], in0=t01[:, JC:],
                        in1=sxxA[:, 2 * JC:3 * JC], op=mybir.AluOpType.add)

            # --- final stats fold ---
            nc.vector.tensor_tensor(
                out=stats8[:, :JC], in0=t012[:, :JC],
                in1=sxA[:, 3 * JC:4 * JC], op=mybir.AluOpType.add)
            nc.vector.tensor_tensor(
                out=stats8[:, JC:], in0=t012[:, JC:],
                in1=sxxA[:, 3 * JC:4 * JC], op=mybir.AluOpType.add)

            # --- combine batch stats across cores (AllGather + local sum) ---
            rstats = constp.tile([128, 2 * JC], F32)
            if os.environ.get("K_SKIP_CC") == "1":
                nc.vector.tensor_copy(out=rstats[:], in_=stats8[:])
            else:
                cc_in = dram.tile([128, 2 * JC], F32)
                cc_out = dram.tile([NCORES, 128, 2 * JC], F32)
                nc.sync.dma_start(out=cc_in[:], in_=stats8[:])
                nc.gpsimd.collective_compute(
                    "AllGather", mybir.AluOpType.bypass,
                    replica_groups=[list(range(NCORES))],
                    ins=[cc_in[:].opt()], outs=[cc_out[:].opt()],
                )
                gstats = constp.tile([128, 2 * JC * NCORES], F32)
                nc.sync.dma_start(
                    out=gstats[:].rearrange("p (i r) -> p i r", r=NCORES),
                    in_=cc_out[:].rearrange("r p i -> p i r"),
                )
                nc.vector.reduce_sum(
                    out=rstats[:].rearrange("p (i o) -> p i o", o=1),
                    in_=gstats[:].rearrange("p (i r) -> p i r", r=NCORES),
                    axis=mybir.AxisListType.X)

            # --- BN coefficients ---
            mean = constp.tile([128, JC], F32)
            nc.vector.tensor_scalar(out=mean[:], in0=rstats[:, :JC],
                                    scalar1=1.0 / B, scalar2=None,
                                    op0=mybir.AluOpType.mult)
            var = constp.tile([128, JC], F32)
            nc.vector.tensor_scalar(out=var[:], in0=rstats[:, JC:],
                                    scalar1=1.0 / B, scalar2=None,
                                    op0=mybir.AluOpType.mult)
            msq = constp.tile([128, JC], F32)
            nc.vector.tensor_tensor(out=msq[:], in0=mean[:], in1=mean[:],
                                    op=mybir.AluOpType.mult)
            nc.vector.tensor_tensor(out=var[:], in0=var[:], in1=msq[:],
                                    op=mybir.AluOpType.subtract)
            # rs = (var + eps) ** -0.5 in one DVE op (keeps ACT on a single
            # func-table set: no mid-kernel table loads)
            rs = constp.tile([128, JC], F32)
            nc.vector.tensor_scalar(out=rs[:], in0=var[:],
                                    scalar1=BN_EPS, scalar2=-0.5,
                                    op0=mybir.AluOpType.add,
                                    op1=mybir.AluOpType.pow)
            scl = constp.tile([128, JC], F32)
            nc.vector.tensor_tensor(out=scl[:], in0=gb_sb[:, :JC],
                                    in1=rs[:], op=mybir.AluOpType.mult)
            shf = constp.tile([128, JC], F32)
            nc.vector.tensor_tensor(out=shf[:], in0=mean[:], in1=scl[:],
                                    op=mybir.AluOpType.mult)
            nc.vector.tensor_tensor(out=shf[:], in0=gb_sb[:, JC:],
                                    in1=shf[:], op=mybir.AluOpType.subtract)

            # --- normalize + relu + fc2 + sigmoid ---
            # relu(scl*x + shf): jc 0,1 on ACT; jc 2,3 on DVE (2 tensor_scalar
            # ops with per-partition scalars) so the two engines overlap.
            po = pout.tile([1, SEGS_PER_CORE], F32, tag="po")
            for jc in range(JC):
                xs = xT[:, jc * SEGS_PER_CORE:(jc + 1) * SEGS_PER_CORE]
                ys = yT[:, jc * SEGS_PER_CORE:(jc + 1) * SEGS_PER_CORE]
                if jc < 2:
                    nc.scalar.activation(
                        out=ys, in_=xs,
                        func=mybir.ActivationFunctionType.Relu,
                        bias=shf[:, jc:jc + 1], scale=scl[:, jc:jc + 1],
                    )
                else:
                    nc.vector.tensor_scalar(
                        out=ys, in0=xs,
                        scalar1=scl[:, jc:jc + 1], scalar2=shf[:, jc:jc + 1],
                        op0=mybir.AluOpType.mult, op1=mybir.AluOpType.add)
                    nc.vector.tensor_scalar(
                        out=ys, in0=ys, scalar1=0.0, scalar2=None,
                        op0=mybir.AluOpType.max)
                nc.tensor.matmul(
                    out=po[:], lhsT=w2t_sb[:, jc:jc + 1], rhs=ys,
                    start=(jc == 0), stop=(jc == JC - 1),
                )
            out_sb = work.tile([1, SEGS_PER_CORE], F32, tag="osb")
            nc.scalar.activation(
                out=out_sb[:], in_=po[:],
                func=mybir.ActivationFunctionType.Sigmoid,
                bias=b2_sb[:1, :1], scale=1.0,
            )
            nc.sync.dma_start(out=out_d[:, :], in_=out_sb[:])

    nc.compile()
    return nc


def _pack_bf16_f32(a16):
    """Pack an ml_dtypes.bfloat16 array (last dim even) into f32 words."""
    u16 = np.ascontiguousarray(a16).view(np.uint16)
    return (u16[..., 0::2].astype(np.uint32)
            | (u16[..., 1::2].astype(np.uint32) << 16)).view(np.float32)


def kernel(tokens, segment_ids, emb, W1, b1, gamma, beta, W2, b2):
    tokens = np.asarray(tokens)
    segment_ids = np.asarray(segment_ids)
    emb = np.asarray(emb, dtype=np.float32)
    W1 = np.asarray(W1, dtype=np.float32)
    gamma = np.asarray(gamma, dtype=np.float32)
    beta = np.asarray(beta, dtype=np.float32)
    W2 = np.asarray(W2, dtype=np.float32)
    b2 = np.asarray(b2, dtype=np.float32)
    # b1 cancels inside BatchNorm; unused.

    L, gsizes, ntiles_total, idx_cols, per_core = _plan(tokens, segment_ids)
    nc = _build(L, gsizes, ntiles_total, idx_cols)

    embp = _pack_bf16_f32(emb.astype(ml_dtypes.bfloat16))

    # w1t[p, hc*H + jc*128 + j] = W1[jc*128 + j, hc*128 + p]
    w1t = np.ascontiguousarray(
        W1.T.reshape(JC, 128, H).transpose(1, 0, 2).reshape(128, JC * H)
    ).astype(ml_dtypes.bfloat16)
    w2t = np.ascontiguousarray(W2.reshape(JC, 128).T).astype(ml_dtypes.bfloat16)
    gb = np.concatenate([gamma.reshape(JC, 128).T,
                         beta.reshape(JC, 128).T], axis=1).astype(np.float32)
    b2h = b2.reshape(1, 1)

    in_maps = []
    for c in range(NCORES):
        in_maps.append({
            "embp": embp,
            "idx16": per_core[c]["idx16"],
            "segsc": per_core[c]["segsc"],
            "recipT": per_core[c]["recipT"],
            "w1t": w1t, "w2t": w2t, "gb": gb, "b2": b2h,
        })

    res = bass_utils.run_bass_kernel_spmd(nc, in_maps, core_ids=list(range(NCORES)))
    out = np.concatenate([res.results[c]["out"].reshape(-1) for c in range(NCORES)])
    return out.reshape(B, 1).astype(np.float32)
